# revision 1
# baseline (speedup 1.0000x reference)
"""Cross-attention (RoPE, H=8, D=64) Trainium2 kernel, 8-core SPMD.

Sharding: core i handles batch b = i//4 and head-pair p = i%4
(heads 2p, 2p+1  ==  channel slice [128p : 128p+128) of the 512-dim space).
Each core computes, for its batch and its 2 heads:
    K.T / Q.T projections (+bias +RoPE)  -> [128, 4096] bf16 (2 heads stacked)
    V projection (+bias)                 -> per-head [128m-chunk, 64] bf16 (+ones col)
    flash-style attention with scores kept transposed (S.T = [m, n]):
        S.T = K.T_h^T-free-chunked matmuls, exp on ScalarE (scale fused),
        AV + softmax denominator in one matmul via the ones column of V',
        normalization via reciprocal + DRAM-roundtrip partition broadcast.
    partial output projection: att.T[slice] @ Wo.T[slice] -> [4096, 512] f32
Host sums the 4 partials per batch and adds bo.

All matmuls run in bf16 (full PE rate); accumulation is fp32 in PSUM.
"""

import sys

if "/opt/trn_rl_repo" not in sys.path:
    sys.path.insert(0, "/opt/trn_rl_repo")

from contextlib import ExitStack

import numpy as np
import ml_dtypes

import concourse.tile as tile
from concourse import bacc, mybir
from concourse.bass_utils import run_bass_kernel_spmd

F32 = mybir.dt.float32
BF16 = mybir.dt.bfloat16
EXP = mybir.ActivationFunctionType.Exp

B, N, C = 2, 4096, 512
H, D = 8, 64
M = 4096
SCALE = float(D) ** -0.5
ROPE_BASE = 10000.0
NCORES = 8
PJ = 128          # channels per core (2 heads)
MB = M // 512     # 8  kv blocks of 512
NB = N // 512     # 8  query blocks of 512
MC = M // 128     # 32 key chunks of 128


def _build(tc, aps):
    nc = tc.nc
    (xT, ctxT, wqT, wkT, wvT, woT, bqT, bkT, bv, cosT, sinT, r2T, scr, out) = aps
    es = ExitStack()
    with es:
        const = es.enter_context(tc.tile_pool(name="const", bufs=1))
        resid = es.enter_context(tc.tile_pool(name="resid", bufs=1))

        # ---- constants ----
        wq_sb = const.tile([128, 4, PJ], BF16)
        nc.sync.dma_start(wq_sb[:], wqT.rearrange("(o p) j -> p o j", p=128))
        wk_sb = const.tile([128, 4, PJ], BF16)
        nc.sync.dma_start(wk_sb[:], wkT.rearrange("(o p) j -> p o j", p=128))
        wv_sb = const.tile([128, 4, PJ], BF16)
        nc.sync.dma_start(wv_sb[:], wvT.rearrange("(o p) j -> p o j", p=128))
        wo_sb = const.tile([128, C], BF16)
        nc.sync.dma_start(wo_sb[:], woT)
        bq_sb = const.tile([128, 1], F32)
        nc.sync.dma_start(bq_sb[:], bqT)
        bk_sb = const.tile([128, 1], F32)
        nc.sync.dma_start(bk_sb[:], bkT)
        bv_sb = const.tile([128, PJ], F32)
        nc.sync.dma_start(bv_sb[:], bv.to_broadcast((128, PJ)))
        r2_sb = const.tile([128, 128], BF16)
        nc.sync.dma_start(r2_sb[:], r2T)
        cos_sb = const.tile([128, N], F32)
        nc.sync.dma_start(cos_sb[:], cosT)
        sin_sb = const.tile([128, N], F32)
        nc.sync.dma_start(sin_sb[:], sinT)

        # ---- residents ----
        KT = resid.tile([128, M], BF16)     # roped K.T, 2 heads stacked on partitions
        Vp0 = resid.tile([128, MC, 65], BF16)  # [m-in-chunk, m-chunk, V|ones]
        Vp1 = resid.tile([128, MC, 65], BF16)
        nc.vector.memset(Vp0[:, :, 64:65], 1.0)
        nc.vector.memset(Vp1[:, :, 64:65], 1.0)

        # ---- shared pools (PSUM budget: ps 2x2 + pv 2 + po 2 = 8 banks) ----
        with (
            tc.tile_pool(name="pw", bufs=3) as work,
            tc.tile_pool(name="ew", bufs=8) as ew,
            tc.tile_pool(name="nw", bufs=2) as nw,
            tc.tile_pool(name="sp", bufs=2, space="PSUM") as sp,
            tc.tile_pool(name="vp", bufs=2, space="PSUM") as vp,
            tc.tile_pool(name="op", bufs=2, space="PSUM") as op,
            tc.tile_pool(name="qp", bufs=3) as qpool,
            tc.tile_pool(name="ap", bufs=2) as apool,
        ):
            def proj_load(src_ap, blk):
                sl = slice(512 * blk, 512 * blk + 512)
                act = work.tile([128, 4, 512], BF16, tag="act")
                nc.sync.dma_start(act[:], src_ap.rearrange("(o p) m -> p o m", p=128)[:, :, sl])
                return act

            def proj_rope(act, w_sb, b_sb, dst, dsl, blk, with_v):
                sl = slice(512 * blk, 512 * blk + 512)
                ps = sp.tile([128, 512], F32, tag="ps")
                for c in range(4):
                    nc.tensor.matmul(ps[:], w_sb[:, c, :], act[:, c, :],
                                     start=(c == 0), stop=(c == 3))
                kb = work.tile([128, 512], BF16, tag="kb")
                nc.vector.tensor_add(out=kb[:], in0=ps[:], in1=b_sb.to_broadcast((128, 512)))
                pr = sp.tile([128, 512], F32, tag="ps")
                nc.tensor.matmul(pr[:], r2_sb[:], kb[:], start=True, stop=True)
                t1 = work.tile([128, 512], F32, tag="t1")
                nc.vector.tensor_mul(out=t1[:], in0=kb[:], in1=cos_sb[:, sl])
                t2 = work.tile([128, 512], F32, tag="t2")
                nc.vector.tensor_mul(out=t2[:], in0=pr[:], in1=sin_sb[:, sl])
                nc.vector.tensor_add(out=dst[:, dsl], in0=t1[:], in1=t2[:])
                if with_v:
                    for mm in range(4):
                        pv = op.tile([128, 128], F32, tag="po")
                        for c in range(4):
                            nc.tensor.matmul(pv[:], act[:, c, 128 * mm:128 * mm + 128],
                                             wv_sb[:, c, :], start=(c == 0), stop=(c == 3))
                        mci = 4 * blk + mm
                        nc.vector.tensor_add(out=Vp0[:, mci, 0:64], in0=pv[:, 0:64],
                                             in1=bv_sb[:, 0:64])
                        nc.vector.tensor_add(out=Vp1[:, mci, 0:64], in0=pv[:, 64:128],
                                             in1=bv_sb[:, 64:128])

            kv_acts = {mb: proj_load(ctxT, mb) for mb in range(2)}
            for mb in range(MB):
                if mb + 2 < MB:
                    kv_acts[mb + 2] = proj_load(ctxT, mb + 2)
                proj_rope(kv_acts.pop(mb), wk_sb, bk_sb, KT,
                          slice(512 * mb, 512 * mb + 512), mb, with_v=True)
            qts = {}
            qts[0] = qpool.tile([128, 512], BF16, tag="qt", name="qt0")
            proj_rope(proj_load(xT, 0), wq_sb, bq_sb, qts[0], slice(0, 512), 0,
                      with_v=False)
            q_acts = {}
            oproj_pending = None

            def emit_oproj(pnb, patt):
                for nn in range(4):
                    rsl = slice(512 * pnb + 128 * nn, 512 * pnb + 128 * nn + 128)
                    po = op.tile([128, 512], F32, tag="po")
                    nc.tensor.matmul(po[:], patt[:, 128 * nn:128 * nn + 128], wo_sb[:],
                                     start=True, stop=True)
                    ob = nw.tile([128, 512], F32, tag="ob")
                    nc.vector.tensor_copy(out=ob[:], in_=po[:])
                    nc.sync.dma_start(out[rsl, :], ob[:])

            # ---- attention + output projection, software-pipelined ----
            for nb in range(NB):
                qt = qts.pop(nb)
                att = apool.tile([128, 512], BF16, tag="att")
                pv0 = vp.tile([128, 512], F32, tag="pv")
                pv1 = vp.tile([128, 512], F32, tag="pv")
                prev = None
                for mc in range(MC + 1):
                    ps01 = None
                    if mc < MC:
                        mcs = slice(128 * mc, 128 * mc + 128)
                        ps01 = sp.tile([128, 1024], F32, tag="ps")
                        nc.tensor.matmul(ps01[:, 0:512], KT[0:64, mcs], qt[0:64, :],
                                         start=True, stop=True, tile_position=(0, 0))
                        nc.tensor.matmul(ps01[:, 512:1024], KT[64:128, mcs], qt[64:128, :],
                                         start=True, stop=True, tile_position=(64, 0))
                    if prev is not None:
                        pmc, pps = prev
                        e01 = ew.tile([128, 1024], BF16, tag="e")
                        nc.scalar.activation(e01[:], pps[:], EXP, scale=SCALE)
                        nc.tensor.matmul(pv0[0:65, :], Vp0[:, pmc, :], e01[:, 0:512],
                                         start=(pmc == 0), stop=(pmc == MC - 1))
                        nc.tensor.matmul(pv1[0:65, :], Vp1[:, pmc, :], e01[:, 512:1024],
                                         start=(pmc == 0), stop=(pmc == MC - 1))
                    # next query block's projection, tucked mid-loop to fill PE idle
                    if mc == 0 and nb + 1 < NB:
                        q_acts[nb + 1] = proj_load(xT, nb + 1)
                    if mc == 6 and nb + 1 < NB:
                        qts[nb + 1] = qpool.tile([128, 512], BF16, tag="qt", name=f"qt{nb+1}")
                        proj_rope(q_acts.pop(nb + 1), wq_sb, bq_sb, qts[nb + 1],
                                  slice(0, 512), nb + 1, with_v=False)
                    if mc == 12 and oproj_pending is not None:
                        emit_oproj(*oproj_pending)
                        oproj_pending = None
                    prev = (mc, ps01) if mc < MC else None
                # normalize:  att = num * (1/denom), denom broadcast via DRAM roundtrip
                rec0 = nw.tile([128, 512], F32, tag="rec0")
                nc.vector.reciprocal(rec0[64:65, :], pv0[64:65, :])
                rec1 = nw.tile([128, 512], F32, tag="rec1")
                nc.vector.reciprocal(rec1[64:65, :], pv1[64:65, :])
                nc.sync.dma_start(scr[2 * nb:2 * nb + 1, :], rec0[64:65, :])
                nc.sync.dma_start(scr[2 * nb + 1:2 * nb + 2, :], rec1[64:65, :])
                bc = nw.tile([128, 512], F32, tag="bc")
                nc.sync.dma_start(bc[0:64, :], scr[2 * nb:2 * nb + 1, :].to_broadcast((64, 512)))
                nc.sync.dma_start(bc[64:128, :], scr[2 * nb + 1:2 * nb + 2, :].to_broadcast((64, 512)))
                nc.vector.tensor_mul(out=att[0:64, :], in0=pv0[0:64, :], in1=bc[0:64, :])
                nc.vector.tensor_mul(out=att[64:128, :], in0=pv1[0:64, :], in1=bc[64:128, :])
                # output projection deferred into the next block's loop
                if nb + 1 < NB:
                    oproj_pending = (nb, att)
                else:
                    emit_oproj(nb, att)


def build_program():
    nc = bacc.Bacc("TRN2", target_bir_lowering=False, debug=False)

    def din(name, shape, dt):
        return nc.dram_tensor(name, shape, dt, kind="ExternalInput").ap()

    aps = (
        din("xT", [C, N], BF16),
        din("ctxT", [C, M], BF16),
        din("wqT", [C, PJ], BF16),
        din("wkT", [C, PJ], BF16),
        din("wvT", [C, PJ], BF16),
        din("woT", [PJ, C], BF16),
        din("bqT", [PJ, 1], F32),
        din("bkT", [PJ, 1], F32),
        din("bv", [1, PJ], F32),
        din("cosT", [PJ, N], F32),
        din("sinT", [PJ, N], F32),
        din("r2T", [PJ, PJ], BF16),
        nc.dram_tensor("scr", [2 * NB, 512], F32).ap(),
        nc.dram_tensor("out", [N, C], F32, kind="ExternalOutput").ap(),
    )
    with tile.TileContext(nc) as tc:
        _build(tc, aps)
    nc.compile()
    return nc


_PROG = None


def _program():
    global _PROG
    if _PROG is None:
        _PROG = build_program()
    return _PROG


def rope_tables():
    idx = np.arange(0, D, 2, dtype=np.float32)
    inv_freq = 1.0 / (ROPE_BASE ** (idx / D))
    t = np.arange(N, dtype=np.float32)
    freqs = t[:, None] * inv_freq[None, :]          # (N, 32)
    emb = np.concatenate([freqs, freqs], axis=1)    # (N, 64)
    cos64 = np.cos(emb).T.astype(np.float32)        # (64, N)
    sin64 = np.sin(emb).T.astype(np.float32)
    cosT = np.ascontiguousarray(np.vstack([cos64, cos64]))
    sinT = np.ascontiguousarray(np.vstack([sin64, sin64]))
    return cosT, sinT


def r2t_matrix():
    R = np.zeros((D, D), np.float32)
    for i in range(D // 2):
        R[2 * i, 2 * i + 1] = -1.0
        R[2 * i + 1, 2 * i] = 1.0
    R2 = np.zeros((PJ, PJ), np.float32)
    R2[0:D, 0:D] = R
    R2[D:PJ, D:PJ] = R
    return np.ascontiguousarray(R2.T).astype(ml_dtypes.bfloat16)


def make_in_maps(x, context, Wq, bq, Wk, bk, Wv, bv, Wo):
    def bf(a):
        return np.ascontiguousarray(a).astype(ml_dtypes.bfloat16)

    def f32c(a):
        return np.ascontiguousarray(a, dtype=np.float32)

    cosT, sinT = rope_tables()
    r2T = r2t_matrix()
    xTb = [bf(x[b].T) for b in range(B)]
    ctxTb = [bf(context[b].T) for b in range(B)]
    in_maps = []
    for core in range(NCORES):
        b, p = core // 4, core % 4
        sl = slice(PJ * p, PJ * p + PJ)
        in_maps.append({
            "xT": xTb[b],
            "ctxT": ctxTb[b],
            "wqT": bf(Wq[sl, :].T),
            "wkT": bf(Wk[sl, :].T),
            "wvT": bf(Wv[sl, :].T),
            "woT": bf(Wo[:, sl].T),
            "bqT": f32c(bq[sl].reshape(PJ, 1)),
            "bkT": f32c(bk[sl].reshape(PJ, 1)),
            "bv": f32c(bv[sl].reshape(1, PJ)),
            "cosT": cosT,
            "sinT": sinT,
            "r2T": r2T,
        })
    return in_maps


def gather(partials, bo):
    final = np.empty((B, N, C), np.float32)
    for b in range(B):
        acc = partials[4 * b].astype(np.float32).copy()
        for p in range(1, 4):
            acc += partials[4 * b + p]
        final[b] = acc + np.asarray(bo, np.float32)[None, :]
    return final


def kernel(x, context, Wq, bq, Wk, bk, Wv, bv, Wo, bo, **kw):
    x = np.asarray(x, np.float32)
    context = np.asarray(context, np.float32)
    nc = _program()
    in_maps = make_in_maps(x, context, np.asarray(Wq, np.float32), np.asarray(bq, np.float32),
                           np.asarray(Wk, np.float32), np.asarray(bk, np.float32),
                           np.asarray(Wv, np.float32), np.asarray(bv, np.float32),
                           np.asarray(Wo, np.float32))
    res = run_bass_kernel_spmd(nc, in_maps, list(range(NCORES)))
    partials = [res.results[i]["out"] for i in range(NCORES)]
    return gather(partials, np.asarray(bo, np.float32))



# revision 18
# speedup vs baseline: 1.2159x; 1.2159x over previous
"""Cross-attention (RoPE, H=8, D=64) Trainium2 kernel, 8-core SPMD.

Sharding: core i handles batch b = i//4 and head-pair p = i%4
(heads 2p, 2p+1  ==  channel slice [128p : 128p+128) of the 512-dim space).

Per core, software-pipelined flash-style attention with transposed scores:
  iteration i:  scores(i) [PE, tile-position pair]
                exp(i-1)  [split: ScalarE exact exp on cols 0:XSPLIT,
                           DVE Schraudolph bit-trick exp on cols XSPLIT:1024]
                AV(i-2)   [PE, 2 matmuls]
  K/V projection+RoPE for ctx blocks 2..7 is interleaved into nb0's loop.
  Normalization is folded into a per-head output projection:
    att numerators copied to SBUF bf16, per-(head,q) denominators via tiny
    ones-matmuls (transposed to partitions), reciprocal, then
    out = (po_h0 * r0 + po_h1 * r1) with per-partition scalars on DVE.
  V bias is folded into the host-side gather (softmax rows sum to 1):
    out += bo + bv @ Wo.T
"""

import sys

if "/opt/trn_rl_repo" not in sys.path:
    sys.path.insert(0, "/opt/trn_rl_repo")

from contextlib import ExitStack

import numpy as np
import ml_dtypes

import concourse.tile as tile
from concourse import bacc, mybir
from concourse.bass_utils import run_bass_kernel_spmd

F32 = mybir.dt.float32
BF16 = mybir.dt.bfloat16
I16 = mybir.dt.int16
EXP = mybir.ActivationFunctionType.Exp
MULT = mybir.AluOpType.mult
ADD = mybir.AluOpType.add

B, N, C = 2, 4096, 512
H, D = 8, 64
M = 4096
SCALE = float(D) ** -0.5
ROPE_BASE = 10000.0
NCORES = 8
PJ = 128          # channels per core (2 heads)
MB = M // 512     # 8  kv blocks of 512
NB = N // 512     # 8  query blocks of 512
MC = M // 128     # 32 key chunks of 128

# ---- exp split: ScalarE handles cols [0:XSPLIT), DVE Schraudolph the rest
XSPLIT = 672
# Schraudolph constants for bf16 bit-pattern exp of (score * SCALE):
#   i16 = score * SCH_A + SCH_B ;  bf16 bits = i16
SCH_A = float(128.0 * np.log2(np.e) * SCALE)
SCH_B = float(16256.0 - 5.25)


def _build(tc, aps):
    nc = tc.nc
    (xT, ctxT, wqT, wkT, wvT, woT, bqT, bkT, cosT, sinT, r2T, out) = aps
    es = ExitStack()
    with es:
        const = es.enter_context(tc.tile_pool(name="const", bufs=1))
        resid = es.enter_context(tc.tile_pool(name="resid", bufs=1))

        # ---- constants ----
        wq_sb = const.tile([128, 4, PJ], BF16)
        nc.sync.dma_start(wq_sb[:], wqT.rearrange("(o p) j -> p o j", p=128))
        wk_sb = const.tile([128, 4, PJ], BF16)
        nc.sync.dma_start(wk_sb[:], wkT.rearrange("(o p) j -> p o j", p=128))
        wv_sb = const.tile([128, 4, PJ], BF16)
        nc.sync.dma_start(wv_sb[:], wvT.rearrange("(o p) j -> p o j", p=128))
        wo_sb = const.tile([128, C], BF16)
        nc.sync.dma_start(wo_sb[:], woT)
        bq_sb = const.tile([128, 1], F32)
        nc.sync.dma_start(bq_sb[:], bqT)
        bk_sb = const.tile([128, 1], F32)
        nc.sync.dma_start(bk_sb[:], bkT)
        r2_sb = const.tile([128, 128], BF16)
        nc.sync.dma_start(r2_sb[:], r2T)
        cos_sb = const.tile([128, N], F32)
        nc.sync.dma_start(cos_sb[:], cosT)
        sin_sb = const.tile([128, N], F32)
        nc.sync.dma_start(sin_sb[:], sinT)
        # ---- residents ----
        KT = resid.tile([128, M], BF16)      # roped K.T, 2 heads on partitions
        # V': per chunk [keys 128, 130] = [V_h0 | ones | V_h1 | ones]
        V = resid.tile([128, MC, 130], BF16)
        nc.vector.memset(V[:, :, 64:65], 1.0)
        nc.vector.memset(V[:, :, 129:130], 1.0)

        with (
            tc.tile_pool(name="kvact", bufs=3) as kvact,
            tc.tile_pool(name="qact", bufs=2) as qact,
            tc.tile_pool(name="work", bufs=4) as work,
            tc.tile_pool(name="ew", bufs=3) as ew,
            tc.tile_pool(name="qp", bufs=2) as qpool,
            tc.tile_pool(name="ap", bufs=2) as apool,
            tc.tile_pool(name="nw", bufs=4) as nw,
            tc.tile_pool(name="rp", bufs=2) as rpool,
            tc.tile_pool(name="sp", bufs=2, space="PSUM") as sp,
            tc.tile_pool(name="vp", bufs=2, space="PSUM") as vp,
            tc.tile_pool(name="op", bufs=2, space="PSUM") as op,
        ):
            def load_block(src_ap, blk, pool):
                sl = slice(512 * blk, 512 * blk + 512)
                act = pool.tile([128, 4, 512], BF16, tag="act")
                nc.sync.dma_start(act[:], src_ap.rearrange("(o p) m -> p o m", p=128)[:, :, sl])
                return act

            def rope(act, w_sb, b_sb, dst, dsl, blk, pool_heavy):
                """projection + RoPE: positions from block `blk`, result into
                dst[:, dsl] (bf16). pool_heavy=True puts the 2 SBUF-only
                elementwise ops on GpSimd."""
                sl = slice(512 * blk, 512 * blk + 512)
                ps = op.tile([128, 512], F32, tag="pp")
                for c in range(4):
                    nc.tensor.matmul(ps[:], w_sb[:, c, :], act[:, c, :],
                                     start=(c == 0), stop=(c == 3))
                kb = work.tile([128, 512], BF16, tag="kb")
                nc.vector.tensor_scalar(out=kb[:], in0=ps[:], scalar1=b_sb[:, 0:1],
                                        scalar2=None, op0=ADD)
                pr = op.tile([128, 512], F32, tag="pp")
                nc.tensor.matmul(pr[:], r2_sb[:], kb[:], start=True, stop=True)
                t1 = work.tile([128, 512], F32, tag="t1")
                eng1 = nc.gpsimd if pool_heavy else nc.vector
                eng1.tensor_tensor(out=t1[:], in0=kb[:], in1=cos_sb[:, sl], op=MULT)
                t2 = work.tile([128, 512], F32, tag="t2")
                nc.vector.tensor_mul(out=t2[:], in0=pr[:], in1=sin_sb[:, sl])
                eng1.tensor_tensor(out=dst[:, dsl], in0=t1[:], in1=t2[:], op=ADD)

            def vproj_chunk(act, blk, mm):
                pv = op.tile([128, 128], F32, tag="pp")
                for c in range(4):
                    nc.tensor.matmul(pv[:], act[:, c, 128 * mm:128 * mm + 128],
                                     wv_sb[:, c, :], start=(c == 0), stop=(c == 3))
                mci = 4 * blk + mm
                nc.vector.tensor_copy(out=V[:, mci, 0:64], in_=pv[:, 0:64])
                nc.vector.tensor_copy(out=V[:, mci, 65:129], in_=pv[:, 64:128])

            # ---- startup: ctx blocks 0..2 prefetch, K/V proj blocks 0..1, q0 ----
            kv_acts = {j: load_block(ctxT, j, kvact) for j in range(3)}
            for j in range(2):
                a = kv_acts[j]
                rope(a, wk_sb, bk_sb, KT, slice(512 * j, 512 * j + 512), j, pool_heavy=True)
                for mm in range(4):
                    vproj_chunk(a, j, mm)
                kv_acts.pop(j)
            q_acts = {0: load_block(xT, 0, qact)}
            qts = {}
            qts[0] = qpool.tile([128, 512], BF16, tag="qt", name="qt0")
            rope(q_acts.pop(0), wq_sb, bq_sb, qts[0], slice(0, 512), 0, pool_heavy=True)

            att_prev = None   # (att_tile, nb_index) awaiting den/oproj
            r_prev = None

            def den_transpose(pv0, pv1):
                """Copy the [1, 512] denominator rows to SBUF, then DMA them
                into [128, 2, 4] = [q-in-chunk, head, chunk] layout."""
                den_sb = rpool.tile([1, 2, 512], F32, tag="dsb")
                nc.vector.tensor_copy(out=den_sb[:, 0, :], in_=pv0[64:65, :])
                nc.vector.tensor_copy(out=den_sb[:, 1, :], in_=pv1[64:65, :])
                denT = rpool.tile([128, 2, 4], F32, tag="den")
                for h in range(2):
                    for c in range(4):
                        nc.sync.dma_start(
                            denT[:, h, c:c + 1],
                            den_sb[:, h, 128 * c:128 * c + 128].rearrange(
                                "one p -> one p ()"))
                return denT

            def emit_oproj_chunk(att, r, pnb, c):
                csl = slice(128 * c, 128 * c + 128)
                rsl = slice(512 * pnb + 128 * c, 512 * pnb + 128 * c + 128)
                po0 = op.tile([128, 512], F32, tag="pp")
                nc.tensor.matmul(po0[:], att[0:64, csl], wo_sb[0:64, :],
                                 start=True, stop=True, tile_position=(0, 0))
                po1 = op.tile([128, 512], F32, tag="pp")
                nc.tensor.matmul(po1[:], att[64:128, csl], wo_sb[64:128, :],
                                 start=True, stop=True, tile_position=(64, 0))
                ob = nw.tile([128, 512], F32, tag="ob")
                nc.vector.tensor_scalar(out=ob[:], in0=po0[:], scalar1=r[:, 0, c:c + 1],
                                        scalar2=None, op0=MULT)
                ob2 = nw.tile([128, 512], F32, tag="ob")
                nc.vector.scalar_tensor_tensor(out=ob2[:], in0=po1[:],
                                               scalar=r[:, 1, c:c + 1],
                                               in1=ob[:], op0=MULT, op1=ADD)
                nc.sync.dma_start(out[rsl, :], ob2[:])

            # ---- attention: 8 query blocks, lag-2 pipeline in each ----
            for nb in range(NB):
                qt = qts.pop(nb)
                pv0 = vp.tile([128, 512], F32, tag="pv")
                pv1 = vp.tile([128, 512], F32, tag="pv")
                ps_t = {}
                e_t = {}
                for i in range(MC + 2):
                    if i < MC:
                        mcs = slice(128 * i, 128 * i + 128)
                        ps = sp.tile([128, 1024], F32, tag="ps")
                        nc.tensor.matmul(ps[:, 0:512], KT[0:64, mcs], qt[0:64, :],
                                         start=True, stop=True, tile_position=(0, 0))
                        nc.tensor.matmul(ps[:, 512:1024], KT[64:128, mcs], qt[64:128, :],
                                         start=True, stop=True, tile_position=(64, 0))
                        ps_t[i] = ps
                    if 0 <= i - 1 < MC:
                        pps = ps_t.pop(i - 1)
                        e = ew.tile([128, 1024], BF16, tag="e")
                        nc.scalar.activation(e[:, 0:XSPLIT], pps[:, 0:XSPLIT], EXP,
                                             scale=SCALE)
                        nc.vector.tensor_scalar(out=e[:, XSPLIT:1024].bitcast(I16),
                                                in0=pps[:, XSPLIT:1024],
                                                scalar1=SCH_A, scalar2=SCH_B,
                                                op0=MULT, op1=ADD)
                        e_t[i - 1] = e
                    if i - 2 >= 0:
                        m = i - 2
                        e2 = e_t.pop(m)
                        nc.tensor.matmul(pv0[0:65, :], V[:, m, 0:65], e2[:, 0:512],
                                         start=(m == 0), stop=(m == MC - 1))
                        nc.tensor.matmul(pv1[0:65, :], V[:, m, 65:130], e2[:, 512:1024],
                                         start=(m == 0), stop=(m == MC - 1))
                    # ---- interleaved work ----
                    if nb == 0:
                        # K/V blocks 2..7: block j processed at iters 4(j-2)..+3
                        j = i // 4 + 2
                        k = i % 4
                        if j <= 7:
                            if k == 0:
                                if j + 1 <= 7:
                                    kv_acts[j + 1] = load_block(ctxT, j + 1, kvact)
                                rope(kv_acts[j], wk_sb, bk_sb, KT, slice(512 * j, 512 * j + 512), j, pool_heavy=True)
                            vproj_chunk(kv_acts[j], j, k)
                            if k == 3:
                                kv_acts.pop(j)
                    else:
                        if i == 2 and att_prev is not None:
                            r_prev = rpool.tile([128, 2, 4], F32, tag="r")
                            nc.vector.reciprocal(r_prev[:], att_prev[2][:])
                        if i in (8, 10, 12, 14) and att_prev is not None:
                            c = (i - 8) // 2
                            emit_oproj_chunk(att_prev[0], r_prev, att_prev[1], c)
                            if c == 3:
                                att_prev = None
                    if i == 18 and nb + 1 < NB:
                        q_acts[nb + 1] = load_block(xT, nb + 1, qact)
                    if i == 24 and nb + 1 < NB:
                        qts[nb + 1] = qpool.tile([128, 512], BF16, tag="qt",
                                                 name=f"qt{nb+1}")
                        rope(q_acts.pop(nb + 1), wq_sb, bq_sb, qts[nb + 1],
                             slice(0, 512), nb + 1, pool_heavy=False)
                # ---- numerators + denominators to SBUF (frees pv) ----
                att = apool.tile([128, 512], BF16, tag="att")
                nc.vector.tensor_copy(out=att[0:64, :], in_=pv0[0:64, :])
                nc.vector.tensor_copy(out=att[64:128, :], in_=pv1[0:64, :])
                denT = den_transpose(pv0, pv1)
                att_prev = (att, nb, denT)

            # ---- tail: last block's den + oproj ----
            r_prev = rpool.tile([128, 2, 4], F32, tag="r")
            nc.vector.reciprocal(r_prev[:], att_prev[2][:])
            for c in range(4):
                emit_oproj_chunk(att_prev[0], r_prev, att_prev[1], c)


def build_program():
    nc = bacc.Bacc("TRN2", target_bir_lowering=False, debug=False)

    def din(name, shape, dt):
        return nc.dram_tensor(name, shape, dt, kind="ExternalInput").ap()

    aps = (
        din("xT", [C, N], BF16),
        din("ctxT", [C, M], BF16),
        din("wqT", [C, PJ], BF16),
        din("wkT", [C, PJ], BF16),
        din("wvT", [C, PJ], BF16),
        din("woT", [PJ, C], BF16),
        din("bqT", [PJ, 1], F32),
        din("bkT", [PJ, 1], F32),
        din("cosT", [PJ, N], F32),
        din("sinT", [PJ, N], F32),
        din("r2T", [PJ, PJ], BF16),
        nc.dram_tensor("out", [N, C], F32, kind="ExternalOutput").ap(),
    )
    with tile.TileContext(nc) as tc:
        _build(tc, aps)
    nc.compile()
    return nc


_PROG = None


def _program():
    global _PROG
    if _PROG is None:
        _PROG = build_program()
    return _PROG


def rope_tables():
    idx = np.arange(0, D, 2, dtype=np.float32)
    inv_freq = 1.0 / (ROPE_BASE ** (idx / D))
    t = np.arange(N, dtype=np.float32)
    freqs = t[:, None] * inv_freq[None, :]          # (N, 32)
    emb = np.concatenate([freqs, freqs], axis=1)    # (N, 64)
    cos64 = np.cos(emb).T.astype(np.float32)        # (64, N)
    sin64 = np.sin(emb).T.astype(np.float32)
    cosT = np.ascontiguousarray(np.vstack([cos64, cos64]))
    sinT = np.ascontiguousarray(np.vstack([sin64, sin64]))
    return cosT, sinT


def r2t_matrix():
    R = np.zeros((D, D), np.float32)
    for i in range(D // 2):
        R[2 * i, 2 * i + 1] = -1.0
        R[2 * i + 1, 2 * i] = 1.0
    R2 = np.zeros((PJ, PJ), np.float32)
    R2[0:D, 0:D] = R
    R2[D:PJ, D:PJ] = R
    return np.ascontiguousarray(R2.T).astype(ml_dtypes.bfloat16)


def make_in_maps(x, context, Wq, bq, Wk, bk, Wv, bv, Wo):
    def bf(a):
        return np.ascontiguousarray(a).astype(ml_dtypes.bfloat16)

    def f32c(a):
        return np.ascontiguousarray(a, dtype=np.float32)

    cosT, sinT = rope_tables()
    r2T = r2t_matrix()
    xTb = [bf(x[b].T) for b in range(B)]
    ctxTb = [bf(context[b].T) for b in range(B)]
    in_maps = []
    for core in range(NCORES):
        b, p = core // 4, core % 4
        sl = slice(PJ * p, PJ * p + PJ)
        in_maps.append({
            "xT": xTb[b],
            "ctxT": ctxTb[b],
            "wqT": bf(Wq[sl, :].T),
            "wkT": bf(Wk[sl, :].T),
            "wvT": bf(Wv[sl, :].T),
            "woT": bf(Wo[:, sl].T),
            "bqT": f32c(bq[sl].reshape(PJ, 1)),
            "bkT": f32c(bk[sl].reshape(PJ, 1)),
            "cosT": cosT,
            "sinT": sinT,
            "r2T": r2T,
        })
    return in_maps


def gather(partials, bo, bv, Wo):
    bo_eff = np.asarray(bo, np.float32) + np.asarray(bv, np.float32) @ np.asarray(Wo, np.float32).T
    final = np.empty((B, N, C), np.float32)
    for b in range(B):
        acc = partials[4 * b].astype(np.float32).copy()
        for p in range(1, 4):
            acc += partials[4 * b + p]
        final[b] = acc + bo_eff[None, :]
    return final


def kernel(x, context, Wq, bq, Wk, bk, Wv, bv, Wo, bo, **kw):
    x = np.asarray(x, np.float32)
    context = np.asarray(context, np.float32)
    nc = _program()
    in_maps = make_in_maps(x, context, np.asarray(Wq, np.float32), np.asarray(bq, np.float32),
                           np.asarray(Wk, np.float32), np.asarray(bk, np.float32),
                           np.asarray(Wv, np.float32), np.asarray(bv, np.float32),
                           np.asarray(Wo, np.float32))
    res = run_bass_kernel_spmd(nc, in_maps, list(range(NCORES)))
    partials = [res.results[i]["out"] for i in range(NCORES)]
    return gather(partials, np.asarray(bo, np.float32), np.asarray(bv, np.float32),
                  np.asarray(Wo, np.float32))


# revision 21
# speedup vs baseline: 1.2254x; 1.0078x over previous
"""Cross-attention (RoPE, H=8, D=64) Trainium2 kernel, 8-core SPMD.

Sharding: core i handles batch b = i//4 and head-pair p = i%4
(heads 2p, 2p+1  ==  channel slice [128p : 128p+128) of the 512-dim space).

Per core, software-pipelined flash-style attention with transposed scores:
  iteration i:  scores(i) [PE, tile-position pair]
                exp(i-1)  [split: ScalarE exact exp on cols 0:XSPLIT,
                           DVE Schraudolph bit-trick exp on cols XSPLIT:1024]
                AV(i-2)   [PE, 2 matmuls]
  K/V projection+RoPE for ctx blocks 2..7 is interleaved into nb0's loop.
  Normalization is folded into a per-head output projection:
    att numerators copied to SBUF bf16, per-(head,q) denominators via tiny
    ones-matmuls (transposed to partitions), reciprocal, then
    out = (po_h0 * r0 + po_h1 * r1) with per-partition scalars on DVE.
  V bias is folded into the host-side gather (softmax rows sum to 1):
    out += bo + bv @ Wo.T
"""

import sys

if "/opt/trn_rl_repo" not in sys.path:
    sys.path.insert(0, "/opt/trn_rl_repo")

from contextlib import ExitStack

import numpy as np
import ml_dtypes

import concourse.tile as tile
from concourse import bacc, mybir
from concourse.bass_utils import run_bass_kernel_spmd

F32 = mybir.dt.float32
BF16 = mybir.dt.bfloat16
I16 = mybir.dt.int16
EXP = mybir.ActivationFunctionType.Exp
MULT = mybir.AluOpType.mult
ADD = mybir.AluOpType.add

B, N, C = 2, 4096, 512
H, D = 8, 64
M = 4096
SCALE = float(D) ** -0.5
ROPE_BASE = 10000.0
NCORES = 8
PJ = 128          # channels per core (2 heads)
MB = M // 512     # 8  kv blocks of 512
NB = N // 512     # 8  query blocks of 512
MC = M // 128     # 32 key chunks of 128

# ---- exp split: ScalarE handles cols [0:XSPLIT), DVE Schraudolph the rest
XSPLIT = 672
# Schraudolph constants for bf16 bit-pattern exp of (score * SCALE):
#   i16 = score * SCH_A + SCH_B ;  bf16 bits = i16
SCH_A = float(128.0 * np.log2(np.e) * SCALE)
SCH_B = float(16256.0 - 5.25)


def _build(tc, aps):
    nc = tc.nc
    (xT, ctxT, wqT, wkT, wvT, woT, bqT, bkT, cosT, sinT, r2T, out) = aps
    es = ExitStack()
    with es:
        const = es.enter_context(tc.tile_pool(name="const", bufs=1))
        resid = es.enter_context(tc.tile_pool(name="resid", bufs=1))

        # ---- constants ----
        wq_sb = const.tile([128, 4, PJ], BF16)
        nc.sync.dma_start(wq_sb[:], wqT.rearrange("(o p) j -> p o j", p=128))
        wk_sb = const.tile([128, 4, PJ], BF16)
        nc.sync.dma_start(wk_sb[:], wkT.rearrange("(o p) j -> p o j", p=128))
        wv_sb = const.tile([128, 4, PJ], BF16)
        nc.sync.dma_start(wv_sb[:], wvT.rearrange("(o p) j -> p o j", p=128))
        wo_sb = const.tile([128, C], BF16)
        nc.sync.dma_start(wo_sb[:], woT)
        bq_sb = const.tile([128, 1], F32)
        nc.sync.dma_start(bq_sb[:], bqT)
        bk_sb = const.tile([128, 1], F32)
        nc.sync.dma_start(bk_sb[:], bkT)
        r2_sb = const.tile([128, 128], BF16)
        nc.sync.dma_start(r2_sb[:], r2T)
        cos_sb = const.tile([128, N], F32)
        nc.sync.dma_start(cos_sb[:], cosT)
        sin_sb = const.tile([128, N], F32)
        nc.sync.dma_start(sin_sb[:], sinT)
        # ---- residents ----
        KT = resid.tile([128, M], BF16)      # roped K.T, 2 heads on partitions
        # V': per chunk [keys 128, 130] = [V_h0 | ones | V_h1 | ones]
        V = resid.tile([128, MC, 130], BF16)
        nc.vector.memset(V[:, :, 64:65], 1.0)
        nc.vector.memset(V[:, :, 129:130], 1.0)

        with (
            tc.tile_pool(name="kvact", bufs=3) as kvact,
            tc.tile_pool(name="qact", bufs=2) as qact,
            tc.tile_pool(name="work", bufs=4) as work,
            tc.tile_pool(name="ew", bufs=3) as ew,
            tc.tile_pool(name="qp", bufs=2) as qpool,
            tc.tile_pool(name="ap", bufs=2) as apool,
            tc.tile_pool(name="nw", bufs=4) as nw,
            tc.tile_pool(name="rp", bufs=2) as rpool,
            tc.tile_pool(name="sp", bufs=2, space="PSUM") as sp,
            tc.tile_pool(name="vp", bufs=2, space="PSUM") as vp,
            tc.tile_pool(name="op", bufs=2, space="PSUM") as op,
        ):
            def load_block(src_ap, blk, pool):
                act = pool.tile([128, 4, 512], BF16, tag="act")
                nc.sync.dma_start(act[:], src_ap[:, blk, :, :])
                return act

            def rope_steps(act, w_sb, b_sb, dst, dsl, blk, pool_heavy):
                """projection + RoPE, split into 4 emission steps to avoid
                PE bursts. positions from block `blk`, result into dst[:, dsl]."""
                sl = slice(512 * blk, 512 * blk + 512)
                eng1 = nc.gpsimd if pool_heavy else nc.vector
                st = {}

                def s0():
                    st["ps"] = op.tile([128, 512], F32, tag="pp", name=f"rps{blk}_{id(st)%997}")
                    for c in range(2):
                        nc.tensor.matmul(st["ps"][:], w_sb[:, c, :], act[:, c, :],
                                         start=(c == 0), stop=False)

                def s1():
                    for c in range(2, 4):
                        nc.tensor.matmul(st["ps"][:], w_sb[:, c, :], act[:, c, :],
                                         start=False, stop=(c == 3))
                    st["kb"] = work.tile([128, 512], BF16, tag="kb", name=f"rkb{blk}_{id(st)%997}")
                    nc.vector.tensor_scalar(out=st["kb"][:], in0=st["ps"][:],
                                            scalar1=b_sb[:, 0:1], scalar2=None,
                                            op0=ADD)

                def s2():
                    st["pr"] = op.tile([128, 512], F32, tag="pp", name=f"rpr{blk}_{id(st)%997}")
                    nc.tensor.matmul(st["pr"][:], r2_sb[:], st["kb"][:],
                                     start=True, stop=True)
                    st["t1"] = work.tile([128, 512], F32, tag="t1", name=f"rt1{blk}_{id(st)%997}")
                    eng1.tensor_tensor(out=st["t1"][:], in0=st["kb"][:],
                                       in1=cos_sb[:, sl], op=MULT)

                def s3():
                    t2 = work.tile([128, 512], F32, tag="t2")
                    nc.vector.tensor_mul(out=t2[:], in0=st["pr"][:], in1=sin_sb[:, sl])
                    eng1.tensor_tensor(out=dst[:, dsl], in0=st["t1"][:], in1=t2[:],
                                       op=ADD)

                return [s0, s1, s2, s3]

            def rope(act, w_sb, b_sb, dst, dsl, blk, pool_heavy):
                for s in rope_steps(act, w_sb, b_sb, dst, dsl, blk, pool_heavy):
                    s()

            def vproj_chunk(act, blk, mm):
                pv = op.tile([128, 128], F32, tag="pp")
                for c in range(4):
                    nc.tensor.matmul(pv[:], act[:, c, 128 * mm:128 * mm + 128],
                                     wv_sb[:, c, :], start=(c == 0), stop=(c == 3))
                mci = 4 * blk + mm
                nc.vector.tensor_copy(out=V[:, mci, 0:64], in_=pv[:, 0:64])
                nc.vector.tensor_copy(out=V[:, mci, 65:129], in_=pv[:, 64:128])

            # ---- startup: ctx blocks 0..2 prefetch, K/V proj blocks 0..1, q0 ----
            kv_acts = {j: load_block(ctxT, j, kvact) for j in range(3)}
            for j in range(2):
                a = kv_acts[j]
                rope(a, wk_sb, bk_sb, KT, slice(512 * j, 512 * j + 512), j,
                     pool_heavy=True)
                for mm in range(4):
                    vproj_chunk(a, j, mm)
                kv_acts.pop(j)
            q_acts = {0: load_block(xT, 0, qact)}
            qts = {}
            qts[0] = qpool.tile([128, 512], BF16, tag="qt", name="qt0")
            rope(q_acts.pop(0), wq_sb, bq_sb, qts[0], slice(0, 512), 0,
                 pool_heavy=True)

            def copy_head(att, den_sb, pv, h):
                """stage one head's numerators + denominator to SBUF and kick
                the 4 transposing den DMAs (gpsimd SWDGE)."""
                nc.vector.tensor_copy(out=att[64 * h:64 * h + 64, :],
                                      in_=pv[0:64, :])
                nc.vector.tensor_copy(out=den_sb[:, h, :], in_=pv[64:65, :])

            def den_dmas(den_sb, denT, h):
                for c in range(4):
                    nc.gpsimd.dma_start(
                        denT[:, h, c:c + 1],
                        den_sb[:, h, 128 * c:128 * c + 128].rearrange(
                            "one p -> one p ()"))

            def oproj_a(att, r, c, st):
                csl = slice(128 * c, 128 * c + 128)
                st["po0"] = op.tile([128, 512], F32, tag="pp", name=f"po0_{c}_{id(st)%997}")
                nc.tensor.matmul(st["po0"][:], att[0:64, csl], wo_sb[0:64, :],
                                 start=True, stop=True, tile_position=(0, 0))
                st["po1"] = op.tile([128, 512], F32, tag="pp", name=f"po1_{c}_{id(st)%997}")
                nc.tensor.matmul(st["po1"][:], att[64:128, csl], wo_sb[64:128, :],
                                 start=True, stop=True, tile_position=(64, 0))
                st["ob"] = nw.tile([128, 512], F32, tag="ob", name=f"ob_{c}_{id(st)%997}")
                nc.vector.tensor_scalar(out=st["ob"][:], in0=st["po0"][:],
                                        scalar1=r[:, 0, c:c + 1], scalar2=None,
                                        op0=MULT)

            def oproj_b(r, pnb, c, st):
                ob2 = nw.tile([128, 512], BF16, tag="ob2")
                nc.vector.scalar_tensor_tensor(out=ob2[:], in0=st["po1"][:],
                                               scalar=r[:, 1, c:c + 1],
                                               in1=st["ob"][:], op0=MULT, op1=ADD)
                rs0 = slice(512 * pnb + 128 * c, 512 * pnb + 128 * c + 64)
                rs1 = slice(512 * pnb + 128 * c + 64, 512 * pnb + 128 * c + 128)
                nc.sync.dma_start(out[rs0, :], ob2[0:64, :])
                nc.sync.dma_start(out[rs1, :], ob2[64:128, :])

            att_prev = None   # (att, nb, den_sb, denT, pv1) awaiting finish
            r_prev = None
            ost = {}

            # ---- attention: 8 query blocks, lag-2 pipeline in each ----
            for nb in range(NB):
                qt = qts.pop(nb)
                pv0 = vp.tile([128, 512], F32, tag="pv")
                pv1 = vp.tile([128, 512], F32, tag="pv")
                ps_t = {}
                e_t = {}
                qsteps = None
                for i in range(MC + 2):
                    if i < MC:
                        mcs = slice(128 * i, 128 * i + 128)
                        ps = sp.tile([128, 1024], F32, tag="ps")
                        nc.tensor.matmul(ps[:, 0:512], KT[0:64, mcs], qt[0:64, :],
                                         start=True, stop=True, tile_position=(0, 0))
                        nc.tensor.matmul(ps[:, 512:1024], KT[64:128, mcs],
                                         qt[64:128, :],
                                         start=True, stop=True, tile_position=(64, 0))
                        ps_t[i] = ps
                    if 0 <= i - 1 < MC:
                        pps = ps_t.pop(i - 1)
                        e = ew.tile([128, 1024], BF16, tag="e")
                        nc.scalar.activation(e[:, 0:XSPLIT], pps[:, 0:XSPLIT], EXP,
                                             scale=SCALE)
                        nc.vector.tensor_scalar(out=e[:, XSPLIT:1024].bitcast(I16),
                                                in0=pps[:, XSPLIT:1024],
                                                scalar1=SCH_A, scalar2=SCH_B,
                                                op0=MULT, op1=ADD)
                        e_t[i - 1] = e
                    if i - 2 >= 0:
                        m = i - 2
                        e2 = e_t.pop(m)
                        nc.tensor.matmul(pv0[0:65, :], V[:, m, 0:65], e2[:, 0:512],
                                         start=(m == 0), stop=(m == MC - 1))
                        nc.tensor.matmul(pv1[0:65, :], V[:, m, 65:130],
                                         e2[:, 512:1024],
                                         start=(m == 0), stop=(m == MC - 1))
                    # ---- interleaved deferred work ----
                    if att_prev is not None:
                        patt, pnb, pden_sb, pdenT, ppv1 = att_prev
                        if i == 0:
                            # second head's numerators + den (frees ppv1)
                            copy_head(patt, pden_sb, ppv1, 1)
                            den_dmas(pden_sb, pdenT, 1)
                        if i == 3:
                            r_prev = rpool.tile([128, 2, 4], F32, tag="r")
                            nc.vector.reciprocal(r_prev[:], pdenT[:])
                        if i in (8, 10, 12, 14):
                            oproj_a(patt, r_prev, (i - 8) // 2, ost)
                        if i in (9, 11, 13, 15):
                            oproj_b(r_prev, pnb, (i - 9) // 2, ost)
                            if i == 15:
                                att_prev = None
                    if nb == 0:
                        # K/V blocks 2..7: block j processed at iters 4(j-2)..+3
                        j = i // 4 + 2
                        k = i % 4
                        if j <= 7:
                            if k == 0:
                                if j + 1 <= 7:
                                    kv_acts[j + 1] = load_block(ctxT, j + 1, kvact)
                                kv_acts[f"rs{j}"] = rope_steps(
                                    kv_acts[j], wk_sb, bk_sb, KT,
                                    slice(512 * j, 512 * j + 512), j,
                                    pool_heavy=True)
                            kv_acts[f"rs{j}"][k]()
                            vproj_chunk(kv_acts[j], j, k)
                            if k == 3:
                                kv_acts.pop(j)
                                kv_acts.pop(f"rs{j}")
                    if i == 18 and nb + 1 < NB:
                        q_acts[nb + 1] = load_block(xT, nb + 1, qact)
                    if i >= 23 and nb + 1 < NB:
                        if i == 23:
                            qts[nb + 1] = qpool.tile([128, 512], BF16, tag="qt",
                                                     name=f"qt{nb+1}")
                            qsteps = rope_steps(q_acts.pop(nb + 1), wq_sb, bq_sb,
                                                qts[nb + 1], slice(0, 512), nb + 1,
                                                pool_heavy=False)
                        if i <= 26:
                            qsteps[i - 23]()
                # ---- first head's numerators + den to SBUF (frees pv0) ----
                att = apool.tile([128, 512], BF16, tag="att")
                den_sb = rpool.tile([1, 2, 512], F32, tag="dsb")
                denT = rpool.tile([128, 2, 4], F32, tag="den")
                copy_head(att, den_sb, pv0, 0)
                den_dmas(den_sb, denT, 0)
                att_prev = (att, nb, den_sb, denT, pv1)

            # ---- tail: last block ----
            patt, pnb, pden_sb, pdenT, ppv1 = att_prev
            copy_head(patt, pden_sb, ppv1, 1)
            den_dmas(pden_sb, pdenT, 1)
            r_prev = rpool.tile([128, 2, 4], F32, tag="r")
            nc.vector.reciprocal(r_prev[:], pdenT[:])
            for c in range(4):
                oproj_a(patt, r_prev, c, ost)
                oproj_b(r_prev, pnb, c, ost)


def build_program():
    nc = bacc.Bacc("TRN2", target_bir_lowering=False, debug=False)

    def din(name, shape, dt):
        return nc.dram_tensor(name, shape, dt, kind="ExternalInput").ap()

    aps = (
        din("xT", [128, NB, 4, 512], BF16),
        din("ctxT", [128, MB, 4, 512], BF16),
        din("wqT", [C, PJ], BF16),
        din("wkT", [C, PJ], BF16),
        din("wvT", [C, PJ], BF16),
        din("woT", [PJ, C], BF16),
        din("bqT", [PJ, 1], F32),
        din("bkT", [PJ, 1], F32),
        din("cosT", [PJ, N], F32),
        din("sinT", [PJ, N], F32),
        din("r2T", [PJ, PJ], BF16),
        nc.dram_tensor("out", [N, C], BF16, kind="ExternalOutput").ap(),
    )
    with tile.TileContext(nc) as tc:
        _build(tc, aps)
    nc.compile()
    return nc


_PROG = None


def _program():
    global _PROG
    if _PROG is None:
        _PROG = build_program()
    return _PROG


def rope_tables():
    idx = np.arange(0, D, 2, dtype=np.float32)
    inv_freq = 1.0 / (ROPE_BASE ** (idx / D))
    t = np.arange(N, dtype=np.float32)
    freqs = t[:, None] * inv_freq[None, :]          # (N, 32)
    emb = np.concatenate([freqs, freqs], axis=1)    # (N, 64)
    cos64 = np.cos(emb).T.astype(np.float32)        # (64, N)
    sin64 = np.sin(emb).T.astype(np.float32)
    cosT = np.ascontiguousarray(np.vstack([cos64, cos64]))
    sinT = np.ascontiguousarray(np.vstack([sin64, sin64]))
    return cosT, sinT


def r2t_matrix():
    R = np.zeros((D, D), np.float32)
    for i in range(D // 2):
        R[2 * i, 2 * i + 1] = -1.0
        R[2 * i + 1, 2 * i] = 1.0
    R2 = np.zeros((PJ, PJ), np.float32)
    R2[0:D, 0:D] = R
    R2[D:PJ, D:PJ] = R
    return np.ascontiguousarray(R2.T).astype(ml_dtypes.bfloat16)


def make_in_maps(x, context, Wq, bq, Wk, bk, Wv, bv, Wo):
    def bf(a):
        return np.ascontiguousarray(a).astype(ml_dtypes.bfloat16)

    def f32c(a):
        return np.ascontiguousarray(a, dtype=np.float32)

    cosT, sinT = rope_tables()
    r2T = r2t_matrix()
    def relayout(a):
        # [N, C] -> [128, nb, 4, 512]: R[p, j, o, m] = a.T[o*128+p, 512j+m]
        aT = np.ascontiguousarray(a.T)              # [512, N]
        R = aT.reshape(4, 128, a.shape[0] // 512, 512).transpose(1, 2, 0, 3)
        return bf(np.ascontiguousarray(R))

    xTb = [relayout(x[b]) for b in range(B)]
    ctxTb = [relayout(context[b]) for b in range(B)]
    in_maps = []
    for core in range(NCORES):
        b, p = core // 4, core % 4
        sl = slice(PJ * p, PJ * p + PJ)
        in_maps.append({
            "xT": xTb[b],
            "ctxT": ctxTb[b],
            "wqT": bf(Wq[sl, :].T),
            "wkT": bf(Wk[sl, :].T),
            "wvT": bf(Wv[sl, :].T),
            "woT": bf(Wo[:, sl].T),
            "bqT": f32c(bq[sl].reshape(PJ, 1)),
            "bkT": f32c(bk[sl].reshape(PJ, 1)),
            "cosT": cosT,
            "sinT": sinT,
            "r2T": r2T,
        })
    return in_maps


def gather(partials, bo, bv, Wo):
    bo_eff = np.asarray(bo, np.float32) + np.asarray(bv, np.float32) @ np.asarray(Wo, np.float32).T
    final = np.empty((B, N, C), np.float32)
    for b in range(B):
        acc = partials[4 * b].astype(np.float32).copy()
        for p in range(1, 4):
            acc += partials[4 * b + p]
        final[b] = acc + bo_eff[None, :]
    return final


def kernel(x, context, Wq, bq, Wk, bk, Wv, bv, Wo, bo, **kw):
    x = np.asarray(x, np.float32)
    context = np.asarray(context, np.float32)
    nc = _program()
    in_maps = make_in_maps(x, context, np.asarray(Wq, np.float32), np.asarray(bq, np.float32),
                           np.asarray(Wk, np.float32), np.asarray(bk, np.float32),
                           np.asarray(Wv, np.float32), np.asarray(bv, np.float32),
                           np.asarray(Wo, np.float32))
    res = run_bass_kernel_spmd(nc, in_maps, list(range(NCORES)))
    partials = [res.results[i]["out"] for i in range(NCORES)]
    return gather(partials, np.asarray(bo, np.float32), np.asarray(bv, np.float32),
                  np.asarray(Wo, np.float32))


# revision 24
# speedup vs baseline: 1.2605x; 1.0287x over previous
"""Cross-attention (RoPE, H=8, D=64) Trainium2 kernel, 8-core SPMD.

Sharding: core i handles batch b = i//4 and head-pair p = i%4
(heads 2p, 2p+1  ==  channel slice [128p : 128p+128) of the 512-dim space).

Per core, software-pipelined flash-style attention with transposed scores:
  iteration i:  scores(i) [PE, tile-position pair]
                exp(i-1)  [split: ScalarE exact exp on cols 0:XSPLIT,
                           DVE Schraudolph bit-trick exp on cols XSPLIT:1024]
                AV(i-2)   [PE, 2 matmuls]
  K/V projection+RoPE for ctx blocks 2..7 is interleaved into nb0's loop.
  Normalization is folded into a per-head output projection:
    att numerators copied to SBUF bf16, per-(head,q) denominators via tiny
    ones-matmuls (transposed to partitions), reciprocal, then
    out = (po_h0 * r0 + po_h1 * r1) with per-partition scalars on DVE.
  V bias is folded into the host-side gather (softmax rows sum to 1):
    out += bo + bv @ Wo.T
"""

import sys

if "/opt/trn_rl_repo" not in sys.path:
    sys.path.insert(0, "/opt/trn_rl_repo")

from contextlib import ExitStack

import numpy as np
import ml_dtypes

import concourse.tile as tile
from concourse import bacc, mybir
from concourse.bass_utils import run_bass_kernel_spmd

F32 = mybir.dt.float32
BF16 = mybir.dt.bfloat16
I16 = mybir.dt.int16
EXP = mybir.ActivationFunctionType.Exp
MULT = mybir.AluOpType.mult
ADD = mybir.AluOpType.add

B, N, C = 2, 4096, 512
H, D = 8, 64
M = 4096
SCALE = float(D) ** -0.5
ROPE_BASE = 10000.0
NCORES = 8
PJ = 128          # channels per core (2 heads)
MB = M // 512     # 8  kv blocks of 512
NB = N // 512     # 8  query blocks of 512
MC = M // 128     # 32 key chunks of 128

# ---- exp split: ScalarE handles cols [0:XSPLIT), DVE Schraudolph the rest
XSPLIT = 672
# Schraudolph constants for bf16 bit-pattern exp of (score * SCALE):
#   i16 = score * SCH_A + SCH_B ;  bf16 bits = i16
SCH_A = float(128.0 * np.log2(np.e) * SCALE)
SCH_B = float(16256.0 - 5.25)


def _build(tc, aps):
    nc = tc.nc
    (xT, ctxT, wqT, wkT, wvT, woT, bqT, bkT, cosT, sinT, r2T, dscr, out) = aps
    es = ExitStack()
    with es:
        const = es.enter_context(tc.tile_pool(name="const", bufs=1))
        resid = es.enter_context(tc.tile_pool(name="resid", bufs=1))

        # ---- constants (order matters: ctx/x blocks race ahead of the
        # big cos/sin tables on shared DMA bandwidth) ----
        wk_sb = const.tile([128, 4, PJ], BF16)
        nc.sync.dma_start(wk_sb[:], wkT.rearrange("(o p) j -> p o j", p=128))
        bk_sb = const.tile([128, 1], F32)
        nc.sync.dma_start(bk_sb[:], bkT)
        wv_sb = const.tile([128, 4, PJ], BF16)
        nc.sync.dma_start(wv_sb[:], wvT.rearrange("(o p) j -> p o j", p=128))
        wq_sb = const.tile([128, 4, PJ], BF16)
        nc.scalar.dma_start(wq_sb[:], wqT.rearrange("(o p) j -> p o j", p=128))
        bq_sb = const.tile([128, 1], F32)
        nc.scalar.dma_start(bq_sb[:], bqT)
        r2_sb = const.tile([128, 128], BF16)
        nc.scalar.dma_start(r2_sb[:], r2T)
        wo_sb = const.tile([128, C], BF16)
        nc.scalar.dma_start(wo_sb[:], woT)
        cos_sb = const.tile([128, N], BF16)
        nc.scalar.dma_start(cos_sb[:], cosT)
        sin_sb = const.tile([128, N], BF16)
        nc.scalar.dma_start(sin_sb[:], sinT)
        # ---- residents ----
        KT = resid.tile([128, M], BF16)      # roped K.T, 2 heads on partitions
        # V': per chunk [keys 128, 130] = [V_h0 | ones | V_h1 | ones]
        V = resid.tile([128, MC, 130], BF16)
        nc.vector.memset(V[:, :, 64:65], 1.0)
        nc.vector.memset(V[:, :, 129:130], 1.0)

        with (
            tc.tile_pool(name="kvact", bufs=8) as kvact,
            tc.tile_pool(name="qact", bufs=2) as qact,
            tc.tile_pool(name="work", bufs=4) as work,
            tc.tile_pool(name="ew", bufs=3) as ew,
            tc.tile_pool(name="qp", bufs=2) as qpool,
            tc.tile_pool(name="ap", bufs=2) as apool,
            tc.tile_pool(name="nw", bufs=4) as nw,
            tc.tile_pool(name="rp", bufs=2) as rpool,
            tc.tile_pool(name="sp", bufs=2, space="PSUM") as sp,
            tc.tile_pool(name="vp", bufs=2, space="PSUM") as vp,
            tc.tile_pool(name="op", bufs=2, space="PSUM") as op,
        ):
            def load_block(src_ap, blk, pool):
                act = pool.tile([128, 4, 512], BF16, tag="act")
                nc.sync.dma_start(act[:], src_ap[:, blk, :, :])
                return act

            def rope_steps(act, w_sb, b_sb, dst, dsl, blk, pool_heavy):
                """projection + RoPE, split into 4 emission steps to avoid
                PE bursts. positions from block `blk`, result into dst[:, dsl]."""
                sl = slice(512 * blk, 512 * blk + 512)
                eng1 = nc.gpsimd if pool_heavy else nc.vector
                st = {}

                def s0():
                    st["ps"] = op.tile([128, 512], F32, tag="pp", name=f"rps{blk}_{id(st)%997}")
                    for c in range(2):
                        nc.tensor.matmul(st["ps"][:], w_sb[:, c, :], act[:, c, :],
                                         start=(c == 0), stop=False)

                def s1():
                    for c in range(2, 4):
                        nc.tensor.matmul(st["ps"][:], w_sb[:, c, :], act[:, c, :],
                                         start=False, stop=(c == 3))
                    st["kb"] = work.tile([128, 512], BF16, tag="kb", name=f"rkb{blk}_{id(st)%997}")
                    nc.vector.tensor_scalar(out=st["kb"][:], in0=st["ps"][:],
                                            scalar1=b_sb[:, 0:1], scalar2=None,
                                            op0=ADD)

                def s2():
                    st["pr"] = op.tile([128, 512], F32, tag="pp", name=f"rpr{blk}_{id(st)%997}")
                    nc.tensor.matmul(st["pr"][:], r2_sb[:], st["kb"][:],
                                     start=True, stop=True)
                    st["t1"] = work.tile([128, 512], F32, tag="t1", name=f"rt1{blk}_{id(st)%997}")
                    eng1.tensor_tensor(out=st["t1"][:], in0=st["kb"][:],
                                       in1=cos_sb[:, sl], op=MULT)

                def s3():
                    t2 = work.tile([128, 512], F32, tag="t2")
                    nc.vector.tensor_mul(out=t2[:], in0=st["pr"][:], in1=sin_sb[:, sl])
                    eng1.tensor_tensor(out=dst[:, dsl], in0=st["t1"][:], in1=t2[:],
                                       op=ADD)

                return [s0, s1, s2, s3]

            def rope(act, w_sb, b_sb, dst, dsl, blk, pool_heavy):
                for s in rope_steps(act, w_sb, b_sb, dst, dsl, blk, pool_heavy):
                    s()

            def vproj_chunk(act, blk, mm):
                pv = op.tile([128, 128], F32, tag="pp")
                for c in range(4):
                    nc.tensor.matmul(pv[:], act[:, c, 128 * mm:128 * mm + 128],
                                     wv_sb[:, c, :], start=(c == 0), stop=(c == 3))
                mci = 4 * blk + mm
                nc.vector.tensor_copy(out=V[:, mci, 0:64], in_=pv[:, 0:64])
                nc.vector.tensor_copy(out=V[:, mci, 65:129], in_=pv[:, 64:128])

            # ---- startup: prefetch all ctx blocks + x0, proj blocks 0..1, q0 ----
            kv_acts = {j: load_block(ctxT, j, kvact) for j in range(2)}
            q_acts = {0: load_block(xT, 0, qact)}
            kv_acts.update({j: load_block(ctxT, j, kvact) for j in range(2, 8)})
            for j in range(2):
                a = kv_acts[j]
                rope(a, wk_sb, bk_sb, KT, slice(512 * j, 512 * j + 512), j,
                     pool_heavy=True)
                for mm in range(4):
                    vproj_chunk(a, j, mm)
                kv_acts.pop(j)
            qts = {}
            qts[0] = qpool.tile([128, 512], BF16, tag="qt", name="qt0")
            rope(q_acts.pop(0), wq_sb, bq_sb, qts[0], slice(0, 512), 0,
                 pool_heavy=True)

            def copy_head(att, den_sb, pv, h):
                """stage one head's numerators + denominator to SBUF and kick
                the 4 transposing den DMAs (gpsimd SWDGE)."""
                nc.vector.tensor_copy(out=att[64 * h:64 * h + 64, :],
                                      in_=pv[0:64, :])
                nc.vector.tensor_copy(out=den_sb[:, h, :], in_=pv[64:65, :])

            def den_dma_out(den_sb, nbi):
                nc.gpsimd.dma_start(dscr[nbi, :, :], den_sb[:, :, :])

            def den_dma_in(denT, nbi):
                nc.gpsimd.dma_start(
                    denT[:], dscr[nbi, :, :].rearrange("h (c p) -> p h c", p=128))

            def oproj_a(att, r, c, st):
                csl = slice(128 * c, 128 * c + 128)
                st["po0"] = op.tile([128, 512], F32, tag="pp", name=f"po0_{c}_{id(st)%997}")
                nc.tensor.matmul(st["po0"][:], att[0:64, csl], wo_sb[0:64, :],
                                 start=True, stop=True, tile_position=(0, 0))
                st["po1"] = op.tile([128, 512], F32, tag="pp", name=f"po1_{c}_{id(st)%997}")
                nc.tensor.matmul(st["po1"][:], att[64:128, csl], wo_sb[64:128, :],
                                 start=True, stop=True, tile_position=(64, 0))
                st["ob"] = nw.tile([128, 512], F32, tag="ob", name=f"ob_{c}_{id(st)%997}")
                nc.vector.tensor_scalar(out=st["ob"][:], in0=st["po0"][:],
                                        scalar1=r[:, 0, c:c + 1], scalar2=None,
                                        op0=MULT)

            def oproj_b(r, pnb, c, st):
                ob2 = nw.tile([128, 512], BF16, tag="ob2")
                nc.vector.scalar_tensor_tensor(out=ob2[:], in0=st["po1"][:],
                                               scalar=r[:, 1, c:c + 1],
                                               in1=st["ob"][:], op0=MULT, op1=ADD)
                rs0 = slice(512 * pnb + 128 * c, 512 * pnb + 128 * c + 64)
                rs1 = slice(512 * pnb + 128 * c + 64, 512 * pnb + 128 * c + 128)
                nc.sync.dma_start(out[rs0, :], ob2[0:64, :])
                nc.sync.dma_start(out[rs1, :], ob2[64:128, :])

            att_prev = None   # (att, nb, den_sb, denT, pv1) awaiting finish
            r_prev = None
            ost = {}

            # ---- attention: 8 query blocks, lag-2 pipeline in each ----
            for nb in range(NB):
                qt = qts.pop(nb)
                pv0 = vp.tile([128, 512], F32, tag="pv")
                pv1 = vp.tile([128, 512], F32, tag="pv")
                ps_t = {}
                e_t = {}
                qsteps = None
                for i in range(MC + 2):
                    if i < MC:
                        mcs = slice(128 * i, 128 * i + 128)
                        ps = sp.tile([128, 1024], F32, tag="ps")
                        nc.tensor.matmul(ps[:, 0:512], KT[0:64, mcs], qt[0:64, :],
                                         start=True, stop=True, tile_position=(0, 0))
                        nc.tensor.matmul(ps[:, 512:1024], KT[64:128, mcs],
                                         qt[64:128, :],
                                         start=True, stop=True, tile_position=(64, 0))
                        ps_t[i] = ps
                    if 0 <= i - 1 < MC:
                        pps = ps_t.pop(i - 1)
                        e = ew.tile([128, 1024], BF16, tag="e")
                        nc.scalar.activation(e[:, 0:XSPLIT], pps[:, 0:XSPLIT], EXP,
                                             scale=SCALE)
                        nc.vector.tensor_scalar(out=e[:, XSPLIT:1024].bitcast(I16),
                                                in0=pps[:, XSPLIT:1024],
                                                scalar1=SCH_A, scalar2=SCH_B,
                                                op0=MULT, op1=ADD)
                        e_t[i - 1] = e
                    if i - 2 >= 0:
                        m = i - 2
                        e2 = e_t.pop(m)
                        nc.tensor.matmul(pv0[0:65, :], V[:, m, 0:65], e2[:, 0:512],
                                         start=(m == 0), stop=(m == MC - 1))
                        nc.tensor.matmul(pv1[0:65, :], V[:, m, 65:130],
                                         e2[:, 512:1024],
                                         start=(m == 0), stop=(m == MC - 1))
                    # ---- interleaved deferred work ----
                    if att_prev is not None:
                        patt, pnb, pden_sb, pdenT, ppv1 = att_prev
                        if i == 0:
                            # second head's numerators + den (frees ppv1)
                            copy_head(patt, pden_sb, ppv1, 1)
                            den_dma_out(pden_sb, pnb)
                        if i == 1:
                            den_dma_in(pdenT, pnb)
                        if i == 4:
                            r_prev = rpool.tile([128, 2, 4], F32, tag="r")
                            nc.vector.reciprocal(r_prev[:], pdenT[:])
                        if i in (8, 10, 12, 14):
                            oproj_a(patt, r_prev, (i - 8) // 2, ost)
                        if i in (9, 11, 13, 15):
                            oproj_b(r_prev, pnb, (i - 9) // 2, ost)
                            if i == 15:
                                att_prev = None
                    if nb == 0:
                        # K/V blocks 2..7: block j processed at iters 4(j-2)..+3
                        j = i // 4 + 2
                        k = i % 4
                        if j <= 7:
                            if k == 0:
                                kv_acts[f"rs{j}"] = rope_steps(
                                    kv_acts[j], wk_sb, bk_sb, KT,
                                    slice(512 * j, 512 * j + 512), j,
                                    pool_heavy=True)
                            kv_acts[f"rs{j}"][k]()
                            vproj_chunk(kv_acts[j], j, k)
                            if k == 3:
                                kv_acts.pop(j)
                                kv_acts.pop(f"rs{j}")
                    if i == 18 and nb + 1 < NB:
                        q_acts[nb + 1] = load_block(xT, nb + 1, qact)
                    if i >= 23 and nb + 1 < NB:
                        if i == 23:
                            qts[nb + 1] = qpool.tile([128, 512], BF16, tag="qt",
                                                     name=f"qt{nb+1}")
                            qsteps = rope_steps(q_acts.pop(nb + 1), wq_sb, bq_sb,
                                                qts[nb + 1], slice(0, 512), nb + 1,
                                                pool_heavy=False)
                        if i <= 26:
                            qsteps[i - 23]()
                # ---- first head's numerators + den to SBUF (frees pv0) ----
                att = apool.tile([128, 512], BF16, tag="att")
                den_sb = rpool.tile([1, 2, 512], F32, tag="dsb")
                denT = rpool.tile([128, 2, 4], F32, tag="den")
                copy_head(att, den_sb, pv0, 0)
                att_prev = (att, nb, den_sb, denT, pv1)

            # ---- tail: last block ----
            patt, pnb, pden_sb, pdenT, ppv1 = att_prev
            copy_head(patt, pden_sb, ppv1, 1)
            den_dma_out(pden_sb, pnb)
            den_dma_in(pdenT, pnb)
            r_prev = rpool.tile([128, 2, 4], F32, tag="r")
            nc.vector.reciprocal(r_prev[:], pdenT[:])
            for c in range(4):
                oproj_a(patt, r_prev, c, ost)
                oproj_b(r_prev, pnb, c, ost)


def build_program():
    nc = bacc.Bacc("TRN2", target_bir_lowering=False, debug=False)

    def din(name, shape, dt):
        return nc.dram_tensor(name, shape, dt, kind="ExternalInput").ap()

    aps = (
        din("xT", [128, NB, 4, 512], BF16),
        din("ctxT", [128, MB, 4, 512], BF16),
        din("wqT", [C, PJ], BF16),
        din("wkT", [C, PJ], BF16),
        din("wvT", [C, PJ], BF16),
        din("woT", [PJ, C], BF16),
        din("bqT", [PJ, 1], F32),
        din("bkT", [PJ, 1], F32),
        din("cosT", [PJ, N], BF16),
        din("sinT", [PJ, N], BF16),
        din("r2T", [PJ, PJ], BF16),
        nc.dram_tensor("dscr", [NB, 2, 512], F32).ap(),
        nc.dram_tensor("out", [N, C], BF16, kind="ExternalOutput").ap(),
    )
    with tile.TileContext(nc) as tc:
        _build(tc, aps)
    nc.compile()
    return nc


_PROG = None


def _program():
    global _PROG
    if _PROG is None:
        _PROG = build_program()
    return _PROG


def rope_tables():
    idx = np.arange(0, D, 2, dtype=np.float32)
    inv_freq = 1.0 / (ROPE_BASE ** (idx / D))
    t = np.arange(N, dtype=np.float32)
    freqs = t[:, None] * inv_freq[None, :]          # (N, 32)
    emb = np.concatenate([freqs, freqs], axis=1)    # (N, 64)
    cos64 = np.cos(emb).T.astype(np.float32)        # (64, N)
    sin64 = np.sin(emb).T.astype(np.float32)
    cosT = np.ascontiguousarray(np.vstack([cos64, cos64]))
    sinT = np.ascontiguousarray(np.vstack([sin64, sin64]))
    return cosT, sinT


def r2t_matrix():
    R = np.zeros((D, D), np.float32)
    for i in range(D // 2):
        R[2 * i, 2 * i + 1] = -1.0
        R[2 * i + 1, 2 * i] = 1.0
    R2 = np.zeros((PJ, PJ), np.float32)
    R2[0:D, 0:D] = R
    R2[D:PJ, D:PJ] = R
    return np.ascontiguousarray(R2.T).astype(ml_dtypes.bfloat16)


def make_in_maps(x, context, Wq, bq, Wk, bk, Wv, bv, Wo):
    def bf(a):
        return np.ascontiguousarray(a).astype(ml_dtypes.bfloat16)

    def f32c(a):
        return np.ascontiguousarray(a, dtype=np.float32)

    cosT, sinT = rope_tables()
    r2T = r2t_matrix()
    def relayout(a):
        # [N, C] -> [128, nb, 4, 512]: R[p, j, o, m] = a.T[o*128+p, 512j+m]
        aT = np.ascontiguousarray(a.T)              # [512, N]
        R = aT.reshape(4, 128, a.shape[0] // 512, 512).transpose(1, 2, 0, 3)
        return bf(np.ascontiguousarray(R))

    xTb = [relayout(x[b]) for b in range(B)]
    ctxTb = [relayout(context[b]) for b in range(B)]
    in_maps = []
    for core in range(NCORES):
        b, p = core // 4, core % 4
        sl = slice(PJ * p, PJ * p + PJ)
        in_maps.append({
            "xT": xTb[b],
            "ctxT": ctxTb[b],
            "wqT": bf(Wq[sl, :].T),
            "wkT": bf(Wk[sl, :].T),
            "wvT": bf(Wv[sl, :].T),
            "woT": bf(Wo[:, sl].T),
            "bqT": f32c(bq[sl].reshape(PJ, 1)),
            "bkT": f32c(bk[sl].reshape(PJ, 1)),
            "cosT": bf(cosT),
            "sinT": bf(sinT),
            "r2T": r2T,
        })
    return in_maps


def gather(partials, bo, bv, Wo):
    bo_eff = np.asarray(bo, np.float32) + np.asarray(bv, np.float32) @ np.asarray(Wo, np.float32).T
    final = np.empty((B, N, C), np.float32)
    for b in range(B):
        acc = partials[4 * b].astype(np.float32).copy()
        for p in range(1, 4):
            acc += partials[4 * b + p]
        final[b] = acc + bo_eff[None, :]
    return final


def kernel(x, context, Wq, bq, Wk, bk, Wv, bv, Wo, bo, **kw):
    x = np.asarray(x, np.float32)
    context = np.asarray(context, np.float32)
    nc = _program()
    in_maps = make_in_maps(x, context, np.asarray(Wq, np.float32), np.asarray(bq, np.float32),
                           np.asarray(Wk, np.float32), np.asarray(bk, np.float32),
                           np.asarray(Wv, np.float32), np.asarray(bv, np.float32),
                           np.asarray(Wo, np.float32))
    res = run_bass_kernel_spmd(nc, in_maps, list(range(NCORES)))
    partials = [res.results[i]["out"] for i in range(NCORES)]
    return gather(partials, np.asarray(bo, np.float32), np.asarray(bv, np.float32),
                  np.asarray(Wo, np.float32))
